# revision 20
# baseline (speedup 1.0000x reference)
"""TRN2 Bass kernel: causal multi-head self-attention block (QKV proj ->
causal softmax attention -> output proj) for B=4, T=2048, C=1024, H=16.

Sharding over 8 NeuronCores: core c handles batch b = c//2 and head-group
g = c%2 (8 of the 16 heads, i.e. 512 of the 1024 hidden channels).  Each
core computes its batch's QKV projection restricted to its head-group's
columns, causal attention for its 8 heads, and a *partial* output
projection (its 512 rows of W_proj).  The host sums the two partial
outputs per batch and adds the host-foldable bias terms
(b_proj, and b_qkv's V part folded through W_proj).

Per-core device kernel layout choices:
  - x is supplied pre-transposed (xT, [C,T]) so the QKV projection needs
    no on-device transpose.
  - Q^T and K^T are produced in [d_head, T] layout, which is exactly the
    operand layout the scores matmul wants (contraction over d on the
    partition axis): S^T blocks [128 k, 512 q] come out of a single
    matmul each, with causal block skipping.
  - exp() runs on the Scalar engine straight out of PSUM (scale=1/8
    folded in); diagonal blocks get a mask strip added first.
  - V carries an extra all-ones column so the softmax denominator drops
    out of the PV matmul for free (row 0 of the PV accumulator).
  - Normalization is a reciprocal + gpsimd partition-broadcast +
    vector multiply, then a small SBUF->SBUF DMA to place the head's
    [64 x 512] output slice at its partition offset in the attn-out
    tiles used as lhsT by the final projection.
"""

import numpy as np
import ml_dtypes

import concourse.bass as bass
import concourse.mybir as mybir
import concourse.tile as tile
from concourse.bass_utils import run_bass_kernel_spmd
from concourse.vector_clock import ScopedClock

# ---------------------------------------------------------------- problem dims
B = 4
T = 2048
C = 1024
H = 16
DH = 64           # head dim
NCORES = 8
HL = H // 2       # heads per core (head-group of 8)
CL = HL * DH      # 512 local channels per core

F32 = mybir.dt.float32
BF16 = mybir.dt.bfloat16
AF = mybir.ActivationFunctionType
NEG = -1.0e9
SCALE = 1.0 / np.sqrt(DH)


class _SplitDrainTileContext(tile.TileContext):
    """TileContext whose tail drain splits its semaphore waits across
    multiple drain instructions; the walrus build in this container
    rejects CTRL instructions carrying more than ~2 sync waits."""

    MAX_WAITS = 1

    def _drain_and_barrier(self, tick_clock, wait_clock):
        nc = self.nc
        drain_inst = nc.sync.drain()
        wait_clock.add_sem_waits(
            drain_inst.ins, ScopedClock({None: tick_clock.global_clock})
        )
        si = drain_inst.ins.sync_info
        waits = list(si.on_wait or []) if si else []
        if len(waits) > self.MAX_WAITS:
            drain_inst.ins.sync_info = mybir.SyncInfo(
                on_wait=waits[: self.MAX_WAITS],
                on_update=list(si.on_update or []),
            )
            rest = waits[self.MAX_WAITS:]
            for i in range(0, len(rest), self.MAX_WAITS):
                extra = nc.sync.drain()
                extra.ins.sync_info = mybir.SyncInfo(
                    on_wait=rest[i : i + self.MAX_WAITS], on_update=[]
                )
        nc.all_engine_barrier()
        assert self.sems is not None
        popped = nc._tile_sem_poison_stack.pop()
        assert popped is self._sem_poison
        nc.clear_and_free_semaphores(list(self.sems.allocated().values()))
        nc.all_engine_barrier()


def _persist(pp, shape, dtype, name):
    return pp.tile(shape, dtype, name=name, tag=name)


_MAX_WAITS = 1


def _split_sync_waits(nc):
    """The walrus build here accepts only a small number of sync waits per
    instruction.  Move excess waits onto InstNoOp wait-carriers inserted
    just before the over-subscribed instruction on the same engine."""
    for bb in nc.main_func.blocks:
        new_insts = []
        changed = False
        for ins in bb.instructions:
            si = ins.sync_info
            waits = list(si.on_wait or []) if si else []
            if len(waits) > _MAX_WAITS:
                changed = True
                extra, keep = waits[_MAX_WAITS:], waits[: _MAX_WAITS]
                for i in range(0, len(extra), _MAX_WAITS):
                    nop = mybir.InstNoOp(name=f"I-waitsplit-{nc.next_id()}")
                    nop.engine = ins.engine
                    nop.sync_info = mybir.SyncInfo(
                        on_wait=extra[i : i + _MAX_WAITS], on_update=[]
                    )
                    new_insts.append(nop)
                ins.sync_info = mybir.SyncInfo(
                    on_wait=keep, on_update=list(si.on_update or [])
                )
            new_insts.append(ins)
        if changed:
            bb.instructions = new_insts


def build_program(loop_reps: int = 1):
    """Build the per-core program.  loop_reps > 1 wraps the whole body in an
    on-device For_i so one NEFF executes the kernel that many times
    (used only for wall-clock timing; the grading path uses 1)."""
    import contextlib

    nc = bass.Bass(trn_type="TRN2")

    xt = nc.dram_tensor("xt", [C, T], BF16, kind="ExternalInput")
    wq = nc.dram_tensor("wq", [C, CL], BF16, kind="ExternalInput")
    wk = nc.dram_tensor("wk", [C, CL], BF16, kind="ExternalInput")
    wv = nc.dram_tensor("wv", [C, CL], BF16, kind="ExternalInput")
    wp = nc.dram_tensor("wp", [CL, C], BF16, kind="ExternalInput")
    bq = nc.dram_tensor("bq", [128, 4], F32, kind="ExternalInput")
    bk = nc.dram_tensor("bk", [128, 4], F32, kind="ExternalInput")
    msk = nc.dram_tensor("msk", [128, 896], BF16, kind="ExternalInput")
    out = nc.dram_tensor("out", [T, C], F32, kind="ExternalOutput")

    NCC = C // 128            # 8 c-chunks of the model dim
    NTS = T // 512            # 4 t-strips
    NTC = T // 128            # 16 t-chunks

    with _SplitDrainTileContext(nc) as tc, tc.tile_pool(
        name="persist", bufs=1
    ) as pp:
        # ------------------------------------------------ persistent SBUF
        xt_sb = [_persist(pp, [128, T], BF16, f"xts{i}") for i in range(NCC)]
        wq_sb = [_persist(pp, [128, CL], BF16, f"wqs{i}") for i in range(NCC)]
        wk_sb = [_persist(pp, [128, CL], BF16, f"wks{i}") for i in range(NCC)]
        wv_sb = [_persist(pp, [128, CL], BF16, f"wvs{i}") for i in range(NCC)]
        wp_sb = [_persist(pp, [128, C], BF16, f"wps{i}") for i in range(CL // 128)]
        bq_sb = _persist(pp, [128, 4], F32, "bqs")
        bk_sb = _persist(pp, [128, 4], F32, "bks")
        msk_sb = _persist(pp, [128, 896], BF16, "msks")
        qt_sb = [_persist(pp, [128, T], BF16, f"qts{p}") for p in range(4)]
        kt_sb = [_persist(pp, [128, T], BF16, f"kts{p}") for p in range(4)]
        # V with a trailing ones column per head: [t-part, t-chunk, head, 64+1]
        v_sb = _persist(pp, [128, NTC, HL, DH + 1], BF16, "vsb")
        aot_sb = [_persist(pp, [128, T], BF16, f"aots{p}") for p in range(4)]

        for _rep in range(loop_reps):
            _emit_body(
                nc, tc, xt, wq, wk, wv, wp, bq, bk, msk, out,
                xt_sb, wq_sb, wk_sb, wv_sb, wp_sb, bq_sb, bk_sb, msk_sb,
                qt_sb, kt_sb, v_sb, aot_sb,
            )
    _split_sync_waits(nc)
    return nc


def _emit_body(
    nc, tc, xt, wq, wk, wv, wp, bq, bk, msk, out,
    xt_sb, wq_sb, wk_sb, wv_sb, wp_sb, bq_sb, bk_sb, msk_sb,
    qt_sb, kt_sb, v_sb, aot_sb,
):
    if True:
        NCC = C // 128
        NTS = T // 512
        for i in range(NCC):
            nc.sync.dma_start(out=wq_sb[i], in_=wq[128 * i : 128 * i + 128, :])
            nc.sync.dma_start(out=xt_sb[i][:, 0:512], in_=xt[128 * i : 128 * i + 128, 0:512])
        nc.sync.dma_start(out=bq_sb, in_=bq[:])
        for i in range(NCC):
            nc.sync.dma_start(out=wk_sb[i], in_=wk[128 * i : 128 * i + 128, :])
        nc.sync.dma_start(out=bk_sb, in_=bk[:])
        nc.sync.dma_start(out=msk_sb, in_=msk[:])
        for i in range(NCC):
            nc.sync.dma_start(out=wv_sb[i], in_=wv[128 * i : 128 * i + 128, :])
        nc.vector.memset(v_sb[:, :, :, DH : DH + 1], 1.0)

        # ------------------------------------------------ pools
        with (
            tc.tile_pool(name="pmm", bufs=5, space="PSUM") as pmm,
            tc.tile_pool(name="po", bufs=3, space="PSUM") as po,
            tc.tile_pool(name="pest", bufs=8) as pest,
            tc.tile_pool(name="pnrm", bufs=4) as pnrm,
            tc.tile_pool(name="pout", bufs=3) as pout,
            tc.tile_pool(name="pdram", bufs=4, space="DRAM") as pdram,
        ):

            def qkv_strip_units(j):
                """QKV^T projection + V for t-strip j, yielded in PE-sized
                units so attention rounds can interleave them as filler."""
                t0 = 512 * j
                if j > 0:
                    for i in range(NCC):
                        nc.sync.dma_start(
                            out=xt_sb[i][:, t0 : t0 + 512],
                            in_=xt[128 * i : 128 * i + 128, t0 : t0 + 512],
                        )
                yield
                for p in range(4):
                    for w_sb, b_sb, o_sb in (
                        (wq_sb, bq_sb, qt_sb),
                        (wk_sb, bk_sb, kt_sb),
                    ):
                        ps = pmm.tile([128, 512], F32, name="psqk", tag="mm")
                        for cc in range(NCC):
                            nc.tensor.matmul(
                                ps,
                                lhsT=w_sb[cc][:, 128 * p : 128 * p + 128],
                                rhs=xt_sb[cc][:, t0 : t0 + 512],
                                start=(cc == 0),
                                stop=(cc == NCC - 1),
                            )
                        nc.vector.tensor_scalar_add(
                            o_sb[p][:, t0 : t0 + 512], ps, b_sb[:, p : p + 1]
                        )
                        yield
                for ic in range(4 * j, 4 * j + 4):
                    psv = pmm.tile([128, 512], F32, name="psv", tag="mm")
                    for cc in range(NCC):
                        nc.tensor.matmul(
                            psv,
                            lhsT=xt_sb[cc][:, 128 * ic : 128 * ic + 128],
                            rhs=wv_sb[cc],
                            start=(cc == 0),
                            stop=(cc == NCC - 1),
                        )
                    nc.vector.tensor_copy(
                        v_sb[:, ic, :, 0:DH],
                        psv.rearrange("p (h d) -> p h d", h=HL),
                    )
                    yield

            def proj_units(j):
                """Partial output projection for the 4 t-chunks of strip j."""
                for qi in range(4 * j, 4 * j + 4):
                    for nh in range(2):
                        ps3 = pmm.tile([128, 512], F32, name="ps3", tag="mm")
                        for cc in range(CL // 128):
                            nc.tensor.matmul(
                                ps3,
                                lhsT=aot_sb[cc][:, 128 * qi : 128 * qi + 128],
                                rhs=wp_sb[cc][:, 512 * nh : 512 * nh + 512],
                                start=(cc == 0),
                                stop=(cc == CL // 128 - 1),
                            )
                        osb = pout.tile([128, 512], F32, name="osb", tag="osb")
                        nc.vector.tensor_copy(osb, ps3)
                        nc.sync.dma_start(
                            out=out[
                                128 * qi : 128 * qi + 128, 512 * nh : 512 * nh + 512
                            ],
                            in_=osb,
                        )
                        yield

            LOOKAHEAD = 3

            def attn(h, j, pump):
                """Causal attention for head h over q-tile j.  The score/exp
                stream is software-pipelined LOOKAHEAD blocks ahead of the PV
                accumulation; diagonal blocks are shortened to their live
                [delta:512] q-range; `pump` is called once per block to emit
                filler projection work that keeps PE busy while the Scalar
                engine works through the exp stream."""
                pt, off = h // 2, (h % 2) * DH
                qt_h = qt_sb[pt][off : off + DH, :]
                kt_h = kt_sb[pt][off : off + DH, :]
                q0 = 512 * j
                nk = 4 * (j + 1)
                pso = po.tile([DH + 1, 512], F32, name="pso", tag="o")
                ests = [None] * nk
                for i in range(nk + LOOKAHEAD):
                    if i < nk:
                        k0 = 128 * i
                        # live q-range of this block (diag blocks shortened)
                        d0 = max(0, k0 - q0)
                        pst = pmm.tile([128, 512], F32, name="pst", tag="mm")
                        nc.tensor.matmul(
                            pst[:, d0:512],
                            lhsT=kt_h[:, k0 : k0 + 128],
                            rhs=qt_h[:, q0 + d0 : q0 + 512],
                            start=True,
                            stop=True,
                        )
                        est = pest.tile([128, 512], BF16, name="est", tag="est")
                        nc.scalar.activation(
                            est[:, d0:512], pst[:, d0:512], AF.Exp, scale=SCALE
                        )
                        if d0 or k0 == q0:
                            # diagonal: zero disallowed (q < k) entries with a
                            # 0/1 strip (bf16, 2x DVE mode)
                            nc.vector.tensor_mul(
                                est[:, d0:512],
                                est[:, d0:512],
                                msk_sb[:, 384 : 896 - d0],
                            )
                        ests[i] = est
                        pump()
                    if i >= LOOKAHEAD:
                        ip = i - LOOKAHEAD
                        d0 = max(0, 128 * ip - q0)
                        nc.tensor.matmul(
                            pso[:, d0:512],
                            lhsT=v_sb[:, ip, h, :],
                            rhs=ests[ip][:, d0:512],
                            start=(ip == 0),
                            stop=(ip == nk - 1),
                        )
                recip = pnrm.tile([DH + 1, 512], F32, name="recip", tag="recip")
                nc.vector.reciprocal(recip[DH : DH + 1, :], pso[DH : DH + 1, :])
                rd = pdram.tile([1, 512], F32, name="rd", tag="rd")
                nc.sync.dma_start(out=rd, in_=recip[DH : DH + 1, :])
                bc = pnrm.tile([DH, 512], F32, name="bc", tag="bc")
                nc.sync.dma_start(out=bc, in_=rd.to_broadcast([DH, 512]))
                stage = pnrm.tile([DH, 512], BF16, name="stage", tag="stage")
                nc.vector.tensor_mul(stage, pso[0:DH, :], bc)
                nc.sync.dma_start(
                    out=aot_sb[pt][off : off + DH, q0 : q0 + 512],
                    in_=stage,
                )

            # j-major schedule: QKV strips and output-projection chunks are
            # threaded through the attention rounds as PE filler.
            for _ in qkv_strip_units(0):
                pass
            for i in range(CL // 128):
                nc.sync.dma_start(out=wp_sb[i], in_=wp[128 * i : 128 * i + 128, :])
            for _ in qkv_strip_units(1):
                pass
            for j in range(NTS):
                gens = []
                if j + 2 < NTS:
                    gens.append(qkv_strip_units(j + 2))
                if j >= 1:
                    gens.append(proj_units(j - 1))
                n_units = (13 if j + 2 < NTS else 0) + (8 if j >= 1 else 0)
                blocks = HL * 4 * (j + 1)
                fill_every = max(1, blocks // (n_units + 1)) if n_units else blocks

                state = {"cnt": 0}

                def pump():
                    state["cnt"] += 1
                    if state["cnt"] % fill_every:
                        return
                    while gens:
                        try:
                            next(gens[0])
                            return
                        except StopIteration:
                            gens.pop(0)

                for h in range(HL):
                    attn(h, j, pump)
                for g in gens:
                    for _ in g:
                        pass
            for _ in proj_units(NTS - 1):
                pass


_PROGRAM = None


def _get_program():
    global _PROGRAM
    if _PROGRAM is None:
        _PROGRAM = build_program()
    return _PROGRAM


def _make_mask_strip():
    # strip[i, c] = 1 where (c - 384) >= i (allowed, q >= k), else 0
    i = np.arange(128)[:, None]
    c = np.arange(896)[None, :]
    return np.where((c - 384) >= i, 1.0, 0.0).astype(ml_dtypes.bfloat16)


def make_in_maps(x, W_qkv, b_qkv, W_proj):
    """Shard the full inputs into the 8 per-core input maps."""
    x = np.asarray(x, dtype=np.float32)
    W_qkv = np.asarray(W_qkv, dtype=np.float32)
    b_qkv = np.asarray(b_qkv, dtype=np.float32)
    W_proj = np.asarray(W_proj, dtype=np.float32)
    bf = ml_dtypes.bfloat16
    strip = _make_mask_strip()
    in_maps = []
    for core in range(NCORES):
        b, g = core // 2, core % 2
        cs = slice(CL * g, CL * g + CL)
        xt = np.ascontiguousarray(x[b].T).astype(bf)
        wq_s = np.ascontiguousarray(W_qkv[:, CL * g : CL * g + CL]).astype(bf)
        wk_s = np.ascontiguousarray(W_qkv[:, C + CL * g : C + CL * g + CL]).astype(bf)
        wv_s = np.ascontiguousarray(
            W_qkv[:, 2 * C + CL * g : 2 * C + CL * g + CL]
        ).astype(bf)
        wp_s = np.ascontiguousarray(W_proj[CL * g : CL * g + CL, :]).astype(bf)
        bq_s = np.ascontiguousarray(b_qkv[cs].reshape(4, 128).T)
        bk_s = np.ascontiguousarray(b_qkv[C + CL * g : C + CL * g + CL].reshape(4, 128).T)
        in_maps.append(
            {
                "xt": xt,
                "wq": wq_s,
                "wk": wk_s,
                "wv": wv_s,
                "wp": wp_s,
                "bq": bq_s,
                "bk": bk_s,
                "msk": strip,
            }
        )
    return in_maps


def gather_output(results, b_qkv, W_proj, b_proj):
    """Sum the per-core partial outputs and fold in the host-side biases."""
    b_qkv = np.asarray(b_qkv, dtype=np.float32)
    W_proj = np.asarray(W_proj, dtype=np.float32)
    b_proj = np.asarray(b_proj, dtype=np.float32)
    bv = b_qkv[2 * C : 3 * C]
    extra = (bv @ W_proj + b_proj).astype(np.float32)
    out = np.empty((B, T, C), dtype=np.float32)
    for b in range(B):
        out[b] = results[2 * b]["out"] + results[2 * b + 1]["out"] + extra
    return out


def kernel(x, W_qkv, b_qkv, W_proj, b_proj):
    nc = _get_program()
    in_maps = make_in_maps(x, W_qkv, b_qkv, W_proj)
    res = run_bass_kernel_spmd(nc, in_maps, list(range(NCORES)))
    return gather_output(res.results, b_qkv, W_proj, b_proj)


# revision 33
# speedup vs baseline: 9.8024x; 9.8024x over previous
"""TRN2 Bass kernel: causal multi-head self-attention block (QKV proj ->
causal softmax attention -> output proj) for B=4, T=2048, C=1024, H=16.

Sharding over 8 NeuronCores: core c handles batch b = c//2 and head-group
g = c%2 (8 of the 16 heads, i.e. 512 of the 1024 hidden channels).  Each
core computes its batch's QKV projection restricted to its head-group's
columns, causal attention for its 8 heads, and a *partial* output
projection (its 512 rows of W_proj).  The host sums the two partial
outputs per batch and adds the host-foldable bias terms
(b_proj, and b_qkv's V part folded through W_proj).

Per-core device kernel layout choices:
  - x is supplied pre-transposed (xT, [C,T]) so the QKV projection needs
    no on-device transpose.
  - Q^T and K^T are produced in [d_head, T] layout, which is exactly the
    operand layout the scores matmul wants (contraction over d on the
    partition axis): S^T blocks [128 k, 512 q] come out of a single
    matmul each, with causal block skipping.
  - exp() runs on the Scalar engine straight out of PSUM (scale=1/8
    folded in); diagonal blocks get a mask strip added first.
  - V carries an extra all-ones column so the softmax denominator drops
    out of the PV matmul for free (row 0 of the PV accumulator).
  - Normalization is a reciprocal + gpsimd partition-broadcast +
    vector multiply, then a small SBUF->SBUF DMA to place the head's
    [64 x 512] output slice at its partition offset in the attn-out
    tiles used as lhsT by the final projection.
"""

import numpy as np
import ml_dtypes

import concourse.bass as bass
import concourse.mybir as mybir
import concourse.tile as tile
from concourse.bass_utils import run_bass_kernel_spmd
from concourse.vector_clock import ScopedClock

# ---------------------------------------------------------------- problem dims
B = 4
T = 2048
C = 1024
H = 16
DH = 64           # head dim
NCORES = 8
HL = H // 2       # heads per core (head-group of 8)
CL = HL * DH      # 512 local channels per core

F32 = mybir.dt.float32
BF16 = mybir.dt.bfloat16
AF = mybir.ActivationFunctionType
NEG = -1.0e9
SCALE = 1.0 / np.sqrt(DH)


class _SplitDrainTileContext(tile.TileContext):
    """TileContext whose tail drain splits its semaphore waits across
    multiple drain instructions; the walrus build in this container
    rejects CTRL instructions carrying more than ~2 sync waits."""

    MAX_WAITS = 1

    def _drain_and_barrier(self, tick_clock, wait_clock):
        nc = self.nc
        drain_inst = nc.sync.drain()
        wait_clock.add_sem_waits(
            drain_inst.ins, ScopedClock({None: tick_clock.global_clock})
        )
        si = drain_inst.ins.sync_info
        waits = list(si.on_wait or []) if si else []
        if len(waits) > self.MAX_WAITS:
            drain_inst.ins.sync_info = mybir.SyncInfo(
                on_wait=waits[: self.MAX_WAITS],
                on_update=list(si.on_update or []),
            )
            rest = waits[self.MAX_WAITS:]
            for i in range(0, len(rest), self.MAX_WAITS):
                extra = nc.sync.drain()
                extra.ins.sync_info = mybir.SyncInfo(
                    on_wait=rest[i : i + self.MAX_WAITS], on_update=[]
                )
        nc.all_engine_barrier()
        assert self.sems is not None
        popped = nc._tile_sem_poison_stack.pop()
        assert popped is self._sem_poison
        nc.clear_and_free_semaphores(list(self.sems.allocated().values()))
        nc.all_engine_barrier()


def _persist(pp, shape, dtype, name):
    return pp.tile(shape, dtype, name=name, tag=name)


_MAX_WAITS = 1


def _split_sync_waits(nc):
    """The walrus build here accepts only a small number of sync waits per
    instruction.  Move excess waits onto InstNoOp wait-carriers inserted
    just before the over-subscribed instruction on the same engine."""
    for bb in nc.main_func.blocks:
        new_insts = []
        changed = False
        for ins in bb.instructions:
            si = ins.sync_info
            waits = list(si.on_wait or []) if si else []
            if len(waits) > _MAX_WAITS:
                changed = True
                extra, keep = waits[_MAX_WAITS:], waits[: _MAX_WAITS]
                for i in range(0, len(extra), _MAX_WAITS):
                    nop = mybir.InstNoOp(name=f"I-waitsplit-{nc.next_id()}")
                    nop.engine = ins.engine
                    nop.sync_info = mybir.SyncInfo(
                        on_wait=extra[i : i + _MAX_WAITS], on_update=[]
                    )
                    new_insts.append(nop)
                ins.sync_info = mybir.SyncInfo(
                    on_wait=keep, on_update=list(si.on_update or [])
                )
            new_insts.append(ins)
        if changed:
            bb.instructions = new_insts


def build_program(loop_reps: int = 1):
    """Build the per-core program.  loop_reps > 1 wraps the whole body in an
    on-device For_i so one NEFF executes the kernel that many times
    (used only for wall-clock timing; the grading path uses 1)."""
    import contextlib

    nc = bass.Bass(trn_type="TRN2")

    xt = nc.dram_tensor("xt", [C, T], BF16, kind="ExternalInput")
    wq = nc.dram_tensor("wq", [C, CL], BF16, kind="ExternalInput")
    wk = nc.dram_tensor("wk", [C, CL], BF16, kind="ExternalInput")
    wv = nc.dram_tensor("wv", [C, CL], BF16, kind="ExternalInput")
    wp = nc.dram_tensor("wp", [CL, C], BF16, kind="ExternalInput")
    bq = nc.dram_tensor("bq", [128, 4], F32, kind="ExternalInput")
    bk = nc.dram_tensor("bk", [128, 4], F32, kind="ExternalInput")
    msk = nc.dram_tensor("msk", [128, 896], BF16, kind="ExternalInput")
    out = nc.dram_tensor("out", [T, C], F32, kind="ExternalOutput")

    NCC = C // 128            # 8 c-chunks of the model dim
    NTS = T // 512            # 4 t-strips
    NTC = T // 128            # 16 t-chunks

    with _SplitDrainTileContext(nc) as tc, tc.tile_pool(
        name="persist", bufs=1
    ) as pp:
        # ------------------------------------------------ persistent SBUF
        xt_sb = [_persist(pp, [128, T], BF16, f"xts{i}") for i in range(NCC)]
        wq_sb = [_persist(pp, [128, CL], BF16, f"wqs{i}") for i in range(NCC)]
        wk_sb = [_persist(pp, [128, CL], BF16, f"wks{i}") for i in range(NCC)]
        wv_sb = [_persist(pp, [128, CL], BF16, f"wvs{i}") for i in range(NCC)]
        wp_sb = [_persist(pp, [128, C], BF16, f"wps{i}") for i in range(CL // 128)]
        bq_sb = _persist(pp, [128, 4], F32, "bqs")
        bk_sb = _persist(pp, [128, 4], F32, "bks")
        msk_sb = _persist(pp, [128, 896], BF16, "msks")
        qt_sb = [_persist(pp, [128, T], BF16, f"qts{p}") for p in range(4)]
        kt_sb = [_persist(pp, [128, T], BF16, f"kts{p}") for p in range(4)]
        ones_sb = _persist(pp, [DH + 1, DH], F32, "ones")
        # V with a trailing ones column per head: [t-part, t-chunk, head, 64+1]
        v_sb = _persist(pp, [128, NTC, HL, DH + 1], BF16, "vsb")
        aot_sb = [
            [_persist(pp, [128, 512], BF16, f"aots{p}_{jj}") for jj in range(T // 512)]
            for p in range(4)
        ]

        for _rep in range(loop_reps):
            _emit_body(
                nc, tc, xt, wq, wk, wv, wp, bq, bk, msk, out,
                xt_sb, wq_sb, wk_sb, wv_sb, wp_sb, bq_sb, bk_sb, msk_sb,
                qt_sb, kt_sb, v_sb, aot_sb, ones_sb,
            )
    _split_sync_waits(nc)
    return nc


def _emit_body(
    nc, tc, xt, wq, wk, wv, wp, bq, bk, msk, out,
    xt_sb, wq_sb, wk_sb, wv_sb, wp_sb, bq_sb, bk_sb, msk_sb,
    qt_sb, kt_sb, v_sb, aot_sb, ones_sb,
):
    if True:
        NCC = C // 128
        NTS = T // 512
        _dmae = [nc.sync, nc.scalar]
        for i in range(NCC):
            _dmae[i % 2].dma_start(out=wq_sb[i], in_=wq[128 * i : 128 * i + 128, :])
            _dmae[(i + 1) % 2].dma_start(
                out=xt_sb[i][:, 0:512], in_=xt[128 * i : 128 * i + 128, 0:512]
            )
        nc.sync.dma_start(out=bq_sb, in_=bq[:])
        for i in range(NCC):
            _dmae[i % 2].dma_start(out=wk_sb[i], in_=wk[128 * i : 128 * i + 128, :])
        nc.sync.dma_start(out=bk_sb, in_=bk[:])
        nc.sync.dma_start(out=msk_sb, in_=msk[:])
        for i in range(NCC):
            _dmae[i % 2].dma_start(out=wv_sb[i], in_=wv[128 * i : 128 * i + 128, :])
        nc.vector.memset(v_sb[:, :, :, DH : DH + 1], 1.0)
        nc.vector.memset(ones_sb, 1.0)

        # ------------------------------------------------ pools
        with (
            tc.tile_pool(name="pmm", bufs=2, space="PSUM") as pmm,
            tc.tile_pool(name="pst", bufs=2, space="PSUM") as pstp,
            tc.tile_pool(name="po", bufs=2, space="PSUM") as po,
            tc.tile_pool(name="pest", bufs=8) as pest,
            tc.tile_pool(name="pnrm", bufs=6) as pnrm,
            tc.tile_pool(name="pout", bufs=4) as pout,
            tc.tile_pool(name="pdram", bufs=8, space="DRAM") as pdram,
        ):

            def qkv_strip_units(j):
                """QKV^T projection + V for t-strip j, yielded in PE-sized
                units so attention rounds can interleave them as filler."""
                t0 = 512 * j
                if j > 0:
                    _de = [nc.sync, nc.gpsimd]
                    for i in range(NCC):
                        _de[i % 2].dma_start(
                            out=xt_sb[i][:, t0 : t0 + 512],
                            in_=xt[128 * i : 128 * i + 128, t0 : t0 + 512],
                        )
                yield
                for p in range(4):
                    for w_sb, b_sb, o_sb in (
                        (wq_sb, bq_sb, qt_sb),
                        (wk_sb, bk_sb, kt_sb),
                    ):
                        ps = pmm.tile([128, 512], F32, name="psqk", tag="mm")
                        for cc in range(NCC):
                            nc.tensor.matmul(
                                ps,
                                lhsT=w_sb[cc][:, 128 * p : 128 * p + 128],
                                rhs=xt_sb[cc][:, t0 : t0 + 512],
                                start=(cc == 0),
                                stop=(cc == NCC - 1),
                            )
                        nc.vector.tensor_scalar_add(
                            o_sb[p][:, t0 : t0 + 512], ps, b_sb[:, p : p + 1]
                        )
                        yield
                for ic in range(4 * j, 4 * j + 4):
                    psv = pmm.tile([128, 512], F32, name="psv", tag="mm")
                    for cc in range(NCC):
                        nc.tensor.matmul(
                            psv,
                            lhsT=xt_sb[cc][:, 128 * ic : 128 * ic + 128],
                            rhs=wv_sb[cc],
                            start=(cc == 0),
                            stop=(cc == NCC - 1),
                        )
                    nc.vector.tensor_copy(
                        v_sb[:, ic, :, 0:DH],
                        psv.rearrange("p (h d) -> p h d", h=HL),
                    )
                    yield

            def proj_units(j):
                """Partial output projection for the 4 t-chunks of strip j."""
                for qi in range(4 * j, 4 * j + 4):
                    for nh in range(2):
                        ps3 = pmm.tile([128, 512], F32, name="ps3", tag="mm")
                        for cc in range(CL // 128):
                            nc.tensor.matmul(
                                ps3,
                                lhsT=aot_sb[cc][qi // 4][
                                    :, 128 * (qi % 4) : 128 * (qi % 4) + 128
                                ],
                                rhs=wp_sb[cc][:, 512 * nh : 512 * nh + 512],
                                start=(cc == 0),
                                stop=(cc == CL // 128 - 1),
                            )
                        osb = pout.tile([128, 512], F32, name="osb", tag="osb")
                        nc.vector.tensor_copy(osb, ps3)
                        nc.sync.dma_start(
                            out=out[
                                128 * qi : 128 * qi + 128, 512 * nh : 512 * nh + 512
                            ],
                            in_=osb,
                        )
                        yield

            LOOKAHEAD = 3

            def attn(h, j, pump, tail=False):
                """Causal attention for head h over q-tile j.  The score/exp
                stream is software-pipelined LOOKAHEAD blocks ahead of the PV
                accumulation; diagonal blocks are shortened to their live
                [delta:512] q-range; `pump` is called once per block to emit
                filler projection work that keeps PE busy while the Scalar
                engine works through the exp stream."""
                pt, off = h // 2, (h % 2) * DH
                qt_h = qt_sb[pt][off : off + DH, :]
                kt_h = kt_sb[pt][off : off + DH, :]
                q0 = 512 * j
                nk = 4 * (j + 1)
                pso = po.tile([DH + 1, 512], F32, name="pso", tag="o")
                ests = [None] * nk
                nfull = 4 * j
                for i in range(nk + LOOKAHEAD):
                    if i < nfull and i % 2 == 0:
                        # paired full blocks: two k-chunks share one 2-bank
                        # PSUM tile and a single exp
                        pst2 = pstp.tile([128, 1024], F32, name="pst2", tag="st")
                        for half in range(2):
                            nc.tensor.matmul(
                                pst2[:, 512 * half : 512 * half + 512],
                                lhsT=kt_h[:, 128 * (i + half) : 128 * (i + half) + 128],
                                rhs=qt_h[:, q0 : q0 + 512],
                                start=True,
                                stop=True,
                            )
                        est2 = pest.tile([128, 1024], BF16, name="est2", tag="est")
                        nc.scalar.activation(est2, pst2, AF.Exp, scale=SCALE)
                        ests[i] = est2[:, 0:512]
                        ests[i + 1] = est2[:, 512:1024]
                        pump()
                    elif i < nfull and i % 2 == 1:
                        pump()  # emitted with partner; keep filler cadence
                    elif i < nk:
                        # diagonal block, shortened to its live q-range
                        k0 = 128 * i
                        d0 = k0 - q0
                        pst2 = pstp.tile([128, 1024], F32, name="pst2", tag="st")
                        nc.tensor.matmul(
                            pst2[:, d0:512],
                            lhsT=kt_h[:, k0 : k0 + 128],
                            rhs=qt_h[:, q0 + d0 : q0 + 512],
                            start=True,
                            stop=True,
                        )
                        est2 = pest.tile([128, 1024], BF16, name="est2", tag="est")
                        nc.scalar.activation(
                            est2[:, d0:512], pst2[:, d0:512], AF.Exp, scale=SCALE
                        )
                        nc.vector.tensor_mul(
                            est2[:, d0:512],
                            est2[:, d0:512],
                            msk_sb[:, 384 : 896 - d0],
                        )
                        ests[i] = est2[:, 0:512]
                        pump()
                    if i >= LOOKAHEAD:
                        ip = i - LOOKAHEAD
                        d0 = max(0, 128 * ip - q0)
                        nc.tensor.matmul(
                            pso[:, d0:512],
                            lhsT=v_sb[:, ip, h, :],
                            rhs=ests[ip][:, d0:512],
                            start=(ip == 0),
                            stop=(ip == nk - 1),
                        )
                recip = pnrm.tile([DH + 1, 512], F32, name="recip", tag="recip")
                nc.vector.reciprocal(recip[DH : DH + 1, :], pso[DH : DH + 1, :])
                bc = pnrm.tile([DH, 512], F32, name="bc", tag="bc")
                if tail:
                    # last head: broadcast the reciprocal row via a PE
                    # outer-product — much shorter latency than the DRAM
                    # round-trip, and PE is idle at this point anyway
                    bcp = po.tile([DH, 512], F32, name="bcp", tag="o")
                    nc.tensor.matmul(
                        bcp,
                        lhsT=ones_sb[DH : DH + 1, :],
                        rhs=recip[DH : DH + 1, :],
                        start=True,
                        stop=True,
                    )
                    nc.scalar.activation(bc, bcp, AF.Copy)
                else:
                    rd = pdram.tile([1, 512], F32, name="rd", tag="rd")
                    nc.sync.dma_start(out=rd, in_=recip[DH : DH + 1, :])
                    nc.sync.dma_start(out=bc, in_=rd.to_broadcast([DH, 512]))
                stage = pnrm.tile([DH, 512], BF16, name="stage", tag="stage")
                nc.vector.tensor_mul(stage, pso[0:DH, :], bc)
                nc.sync.dma_start(out=aot_sb[pt][j][off : off + DH, :], in_=stage)

            # j-major schedule: QKV strips and output-projection chunks are
            # threaded through the attention rounds as PE filler, weighted
            # toward the later (bigger, exp-bound) rounds.
            for _ in qkv_strip_units(0):
                pass
            for i in range(CL // 128):
                nc.sync.dma_start(out=wp_sb[i], in_=wp[128 * i : 128 * i + 128, :])
            FILLER_PLAN = {0: [1], 1: [2], 2: [3, -1], 3: [-2, -3]}
            for j in range(NTS):
                gens = []
                n_units = 0
                for f in FILLER_PLAN[j]:
                    if f >= 0:
                        gens.append(qkv_strip_units(f))
                        n_units += 13
                    else:
                        gens.append(proj_units(-f - 1))
                        n_units += 8
                blocks = HL * 4 * (j + 1)
                # in the last round, hold back a few filler units to run
                # after the final attention block, covering the last head's
                # normalize-chain latency before the output projection
                reserve = 6 if j == NTS - 1 else 0
                usable = max(1, n_units - reserve)
                pump_calls = HL * (4 * (j + 1) + LOOKAHEAD)
                fill_every = (
                    max(1, (pump_calls - 8) // (usable + 1)) if n_units else blocks
                )

                def _advance():
                    while gens:
                        try:
                            next(gens[0])
                            return
                        except StopIteration:
                            gens.pop(0)

                _advance()  # strip DMAs (unit 0) get a head start
                state = {"cnt": 0, "used": 0}

                def pump():
                    state["cnt"] += 1
                    # let the strip's xt DMAs land before filler matmuls;
                    # stop at the usable budget so `reserve` units remain
                    # for the post-round drain
                    if state["cnt"] < 8 or (state["cnt"] - 8) % fill_every:
                        return
                    if state["used"] >= usable:
                        return
                    state["used"] += 1
                    _advance()

                for h in range(HL):
                    attn(h, j, pump, tail=(j == NTS - 1 and h == HL - 1))
                for g in gens:
                    for _ in g:
                        pass
            for _ in proj_units(NTS - 1):
                pass


_PROGRAM = None


def _get_program():
    global _PROGRAM
    if _PROGRAM is None:
        _PROGRAM = build_program()
    return _PROGRAM


def _make_mask_strip():
    # strip[i, c] = 1 where (c - 384) >= i (allowed, q >= k), else 0
    i = np.arange(128)[:, None]
    c = np.arange(896)[None, :]
    return np.where((c - 384) >= i, 1.0, 0.0).astype(ml_dtypes.bfloat16)


def make_in_maps(x, W_qkv, b_qkv, W_proj):
    """Shard the full inputs into the 8 per-core input maps."""
    x = np.asarray(x, dtype=np.float32)
    W_qkv = np.asarray(W_qkv, dtype=np.float32)
    b_qkv = np.asarray(b_qkv, dtype=np.float32)
    W_proj = np.asarray(W_proj, dtype=np.float32)
    bf = ml_dtypes.bfloat16
    strip = _make_mask_strip()
    in_maps = []
    for core in range(NCORES):
        b, g = core // 2, core % 2
        cs = slice(CL * g, CL * g + CL)
        xt = np.ascontiguousarray(x[b].T).astype(bf)
        wq_s = np.ascontiguousarray(W_qkv[:, CL * g : CL * g + CL]).astype(bf)
        wk_s = np.ascontiguousarray(W_qkv[:, C + CL * g : C + CL * g + CL]).astype(bf)
        wv_s = np.ascontiguousarray(
            W_qkv[:, 2 * C + CL * g : 2 * C + CL * g + CL]
        ).astype(bf)
        wp_s = np.ascontiguousarray(W_proj[CL * g : CL * g + CL, :]).astype(bf)
        bq_s = np.ascontiguousarray(b_qkv[cs].reshape(4, 128).T)
        bk_s = np.ascontiguousarray(b_qkv[C + CL * g : C + CL * g + CL].reshape(4, 128).T)
        in_maps.append(
            {
                "xt": xt,
                "wq": wq_s,
                "wk": wk_s,
                "wv": wv_s,
                "wp": wp_s,
                "bq": bq_s,
                "bk": bk_s,
                "msk": strip,
            }
        )
    return in_maps


def gather_output(results, b_qkv, W_proj, b_proj):
    """Sum the per-core partial outputs and fold in the host-side biases."""
    b_qkv = np.asarray(b_qkv, dtype=np.float32)
    W_proj = np.asarray(W_proj, dtype=np.float32)
    b_proj = np.asarray(b_proj, dtype=np.float32)
    bv = b_qkv[2 * C : 3 * C]
    extra = (bv @ W_proj + b_proj).astype(np.float32)
    out = np.empty((B, T, C), dtype=np.float32)
    for b in range(B):
        out[b] = results[2 * b]["out"] + results[2 * b + 1]["out"] + extra
    return out


def kernel(x, W_qkv, b_qkv, W_proj, b_proj):
    nc = _get_program()
    in_maps = make_in_maps(x, W_qkv, b_qkv, W_proj)
    res = run_bass_kernel_spmd(nc, in_maps, list(range(NCORES)))
    return gather_output(res.results, b_qkv, W_proj, b_proj)
